# revision 2
# baseline (speedup 1.0000x reference)
"""Trainium2 Bass kernel for nn_CcLoss (gnn_message_passing) — v2.

Full inputs: features [64, 1024, 128] f32, tau scalar f32.
Data-parallel over batch B across 8 NeuronCores (8 samples per core).

v2 design notes (vs v1):
- All heavy elementwise moved off GpSimd (Pool) onto Act/DVE at their
  native rates; Pool keeps only [128,8]-sized smalls.
- proto matmul runs bf16 mask x bf16 f16h (no fp8 hi/lo split).
- One-sided normalization: sim psum[p,q] = <f_p/2, fn_q> compared against
  taur_p = (tau/2)*r_p (per-partition threshold), so only one transposed
  tensor family is built per sample.
- fT_h (= transpose of f/2, bf16) produced by ONE dma_start_transpose per
  sample reading a DRAM staging copy of f16h (64 XBAR tiles per instr)
  instead of 8 per-tile SBUF transposes.
- fnT = fT_h * rinv2_rep (rinv2 = 2/r broadcast along partitions via the
  row-replicate DRAM round trip).
- Sign-trick on Act tiles (S in {-1,+1}) and is_gt*2 on DVE tiles
  (mask2 in {0,2}) against f16h = f/2 stationary:
     proto[q,d] = pT2[d,q] + c_col[d],  c_col = colsum of f16h over Act tiles
     deg2 = 2*deg = dacc + 1024*[tile is Act-tile]
     protoS = (pT2 + c_col) * rrep,  rrep = rep(2/deg2)
- Stats accumulate straight into the output tile:
    cols 0:8 ff per-tile, 8: pf2 (= pf/2), 9: pp, 10: gtsum.
Host combines stats into MSE + Pearson loss (exact algebra of reference).
"""

import numpy as np

B, P, D = 64, 1024, 128
NCORES = 8
BLOC = B // NCORES          # samples per core
NT = P // 128               # 128-row tiles per sample
SW = 12                     # stat cols: 0:8 ff_t, 8 pf2, 9 ppA, 10 ppB, 11 gt

# Threshold engine split: sim row-tiles 0..NA-1 on Act (Sign form),
# NA..7 on DVE (is_gt * 2 -> {0,2}).
NA = 5
# sim row-tile processing order interleaves Act/DVE thresholds
MT_ORDER = (0, 5, 1, 6, 2, 7, 3, 4)
# which engine materializes pp (Square protoS): "dve" or "act"
PP_ENGINE = "dve"

_PROG = None


def _build_program():
    import concourse.tile as tile
    from concourse import bacc, mybir, masks

    f32 = mybir.dt.float32
    bf16 = mybir.dt.bfloat16
    AF = mybir.ActivationFunctionType
    OP = mybir.AluOpType

    nc = bacc.Bacc(
        "TRN2",
        target_bir_lowering=False,
        debug=False,
        enable_asserts=False,
        num_devices=NCORES,
    )
    feats = nc.dram_tensor("features", [BLOC, P, D], f32, kind="ExternalInput").ap()
    tau_d = nc.dram_tensor("tau", [1, 1], f32, kind="ExternalInput").ap()
    oh_d = nc.dram_tensor("onehot", [8, NT * 128], bf16, kind="ExternalInput").ap()
    out_d = nc.dram_tensor("out", [128, BLOC * SW], f32, kind="ExternalOutput").ap()
    # DRAM staging: f16h per sample (p-major) for the big XBAR transpose
    f16d = nc.dram_tensor("f16_scratch", [BLOC, P, D], bf16, kind="Internal").ap()
    # row scratch for partition-replication round trips (rinv2 / rdeg2)
    rrow_d = nc.dram_tensor("rrow_scratch", [BLOC, 2, P], bf16, kind="Internal").ap()

    with tile.TileContext(nc) as tc:
        from contextlib import ExitStack

        with ExitStack() as ctx:
            const = ctx.enter_context(tc.tile_pool(name="const", bufs=1))
            fpool = ctx.enter_context(tc.tile_pool(name="f", bufs=3))
            sqpool = ctx.enter_context(tc.tile_pool(name="sq", bufs=2))
            fhpool = ctx.enter_context(tc.tile_pool(name="f16h", bufs=5))
            ftpool = ctx.enter_context(tc.tile_pool(name="fTh", bufs=4))
            fnpool = ctx.enter_context(tc.tile_pool(name="fnT", bufs=3))
            mpool = ctx.enter_context(tc.tile_pool(name="mask", bufs=2))
            pspool = ctx.enter_context(tc.tile_pool(name="protoS", bufs=2))
            gpool = ctx.enter_context(tc.tile_pool(name="gscr", bufs=4))
            smpool = ctx.enter_context(tc.tile_pool(name="small", bufs=8))
            rowpool = ctx.enter_context(tc.tile_pool(name="rows", bufs=4))
            reppool = ctx.enter_context(tc.tile_pool(name="reps", bufs=4))
            pss_pool = ctx.enter_context(tc.tile_pool(name="pss", bufs=2, space="PSUM"))
            pT_pool = ctx.enter_context(tc.tile_pool(name="pT", bufs=1, space="PSUM"))
            prow_pool = ctx.enter_context(tc.tile_pool(name="prow", bufs=2, space="PSUM"))

            ident16 = const.tile([128, 128], bf16)
            masks.make_identity(nc, ident16[:])
            ident32 = const.tile([128, 128], f32)
            masks.make_identity(nc, ident32[:])
            tau_bc = const.tile([128, 1], f32)
            nc.sync.dma_start(tau_bc[:], tau_d[0, :].partition_broadcast(128))
            statall = const.tile([128, BLOC * SW], f32)
            twos = const.tile([128, P], bf16)
            nc.vector.memset(twos[:], 2.0)
            # one-hot rows for PE row-broadcast (fill/drain path)
            onehot = const.tile([8, NT * 128], bf16)
            nc.sync.dma_start(onehot[:], oh_d)
            # prefetch activation tables (Square/Sqrt/Sign) so the ~2.7us
            # table loads don't land mid-pipeline
            warm = const.tile([128, 1], f32)
            nc.scalar.activation(warm[:], tau_bc[:], AF.Square)
            nc.scalar.activation(warm[:], tau_bc[:], AF.Sqrt)
            nc.scalar.activation(warm[:], tau_bc[:], AF.Sign)

            st = {}

            def row_replicate_pre(src32, tag):
                """[128,8] f32 -> [8,128] bf16 row of 2*src via PE f32
                transpose + small DVE copy (no Act in the chain)."""
                prow = prow_pool.tile([8, 128], f32, tag="rowT")
                nc.tensor.matmul(prow[:], src32[:], ident32[:], is_transpose=True)
                row8 = rowpool.tile([8, 128], bf16, tag=f"{tag}r8")
                nc.vector.tensor_scalar_mul(row8[:], prow[:], 2.0)
                return row8

            def row_replicate_psum(row8, tag):
                """[8,128] bf16 row -> [128,P] bf16 replicated via 8 K=8
                one-hot matmuls into PSUM + Act copy. No DRAM latency; used
                for fill/drain samples where engines are otherwise idle."""
                rps = pss_pool.tile([128, P], f32, tag="pss")
                for t in range(NT):
                    nc.tensor.matmul(
                        rps[:, t * 128 : (t + 1) * 128],
                        onehot[:, t * 128 : (t + 1) * 128],
                        row8[:],
                        start=True,
                        stop=True,
                    )
                rep = reppool.tile([128, P], bf16, tag=f"{tag}rep")
                nc.scalar.copy(rep[:], rps[:])
                return rep

            def row_replicate_dma(row8, s, slot, tag):
                """Store the row to DRAM and broadcast-load along
                partitions (emitted late so SP never waits on it)."""
                nc.sync.dma_start(
                    rrow_d[s, slot].rearrange("(t p) -> t p", t=NT), row8[:]
                )
                rep = reppool.tile([128, P], bf16, tag=f"{tag}rep")
                nc.sync.dma_start(rep[:], rrow_d[s, slot].partition_broadcast(128))
                return rep

            def stage_load(s):
                fb = fpool.tile([128, NT * 128], f32, tag="fb")
                nc.sync.dma_start(
                    fb[:].rearrange("p (t d) -> p t d", t=NT),
                    feats[s].rearrange("(t p) d -> p t d", p=128),
                )
                st[s] = {"fb": fb}

            def stage_prep(s):
                v = st[s]
                fb = v["fb"]
                ffcols = statall[:, s * SW : s * SW + 8]
                # row norms^2 per tile: Act Square + DVE per-tile reduce
                sq = sqpool.tile([128, NT * 128], f32, tag="sq")
                nc.scalar.activation(sq[:], fb[:], AF.Square)
                nc.vector.tensor_reduce(
                    ffcols,
                    sq[:].rearrange("p (t d) -> p t d", t=NT),
                    axis=mybir.AxisListType.X,
                    op=OP.add,
                )
                sroot = smpool.tile([128, 8], f32, tag="sroot")
                nc.scalar.activation(sroot[:], ffcols, AF.Sqrt)
                rinv = smpool.tile([128, 8], f32, tag="rinv")
                nc.vector.reciprocal(rinv[:], sroot[:])
                # taur = (tau/2)*r ; ntaur = -(tau/2)*r   [Pool]
                taur = smpool.tile([128, 8], f32, tag="taur")
                nc.gpsimd.tensor_scalar(
                    taur[:], sroot[:], tau_bc[:], 0.5, op0=OP.mult, op1=OP.mult
                )
                ntaur = smpool.tile([128, 8], f32, tag="ntaur")
                nc.gpsimd.tensor_scalar_mul(ntaur[:], taur[:], -1.0)

                # f16h = 0.5*f in bf16 (DVE, 2x_2P) and stage to DRAM
                f16h = fhpool.tile([128, NT * 128], bf16, tag="f16h")
                nc.vector.tensor_scalar_mul(f16h[:], fb[:], 0.5)
                nc.sync.dma_start(
                    f16d[s].rearrange("(t p) d -> p t d", p=128),
                    f16h[:].rearrange("p (t d) -> p t d", t=NT),
                )
                row = row_replicate_pre(rinv, "ri")
                if s < 2:
                    v["rrep"] = row_replicate_psum(row, "ri")
                else:
                    v["rrep"] = row_replicate_dma(row, s, 0, "ri")
                v.update(f16h=f16h, taur=taur, ntaur=ntaur)

            def stage_simx_head(s):
                v = st[s]
                # one big XBAR transpose from DRAM: [P, D] -> [128(d), P]
                fTh = ftpool.tile([128, P], bf16, tag="fTh")
                nc.sync.dma_start_transpose(fTh[:], f16d[s])
                # fnT = fT_h * rinv2_rep  (bf16 TT)
                fnT = fnpool.tile([128, P], bf16, tag="fnT")
                nc.vector.tensor_tensor(
                    fnT[:], fTh[:], v["rrep"][:], op=OP.mult
                )
                v.update(fTh=fTh, fnT=fnT)

            def stage_sim(s):
                v = st[s]
                fTh, fnT = v["fTh"], v["fnT"]
                taur, ntaur = v["taur"], v["ntaur"]
                mask_t = mpool.tile([128, NT * P], bf16, tag="mask")
                dacc = smpool.tile([128, 8], f32, tag="dacc")
                pT = pT_pool.tile([128, P], f32, tag="pT")
                v["pT"] = pT
                mk = mask_t[:].rearrange("p (k q) -> p k q", k=NT)
                f16h = v["f16h"].rearrange("p (k d) -> p k d", k=NT)
                for mt in MT_ORDER:
                    pss = pss_pool.tile([128, 1024], f32, tag="pss")
                    for nb in range(2):
                        nc.tensor.matmul(
                            pss[:, nb * 512 : (nb + 1) * 512],
                            fTh[:, mt * 128 : (mt + 1) * 128],
                            fnT[:, nb * 512 : (nb + 1) * 512],
                            start=True,
                            stop=True,
                        )
                    blk = mask_t[:, mt * P : (mt + 1) * P]
                    if mt < NA:
                        # S in {-1,+1}; dacc = 2*deg - 1024
                        nc.scalar.activation(
                            blk, pss[:], AF.Sign, bias=ntaur[:, mt : mt + 1],
                            accum_out=dacc[:, mt : mt + 1],
                        )
                    else:
                        # {0,2}; dacc = 2*deg  (stt with const-2 tensor: the
                        # dual-imm tensor_scalar path miscomputes max elements)
                        nc.vector.scalar_tensor_tensor(
                            blk, pss[:], taur[:, mt : mt + 1], twos[:],
                            op0=OP.is_gt, op1=OP.mult,
                            accum_out=dacc[:, mt : mt + 1],
                        )
                # protoT accumulation as one contiguous PE group: proto-k
                # only needs mask-k, all long since ready -> dense MM burst
                for done, mt in enumerate(MT_ORDER):
                    for nb in range(2):
                        nc.tensor.matmul(
                            pT[:, nb * 512 : (nb + 1) * 512],
                            f16h[:, mt, :],
                            mk[:, mt, nb * 512 : (nb + 1) * 512],
                            start=(done == 0),
                            stop=(done == 7),
                            skip_group_check=True,
                        )

                # deg2 = 2*deg: Act tiles need +1024  [Pool]
                deg2 = smpool.tile([128, 8], f32, tag="deg2")
                nc.gpsimd.tensor_scalar(
                    deg2[:, 0:NA], dacc[:, 0:NA], 1.0, 1024.0,
                    op0=OP.mult, op1=OP.add,
                )
                nc.gpsimd.tensor_copy(deg2[:, NA:8], dacc[:, NA:8])
                rec = smpool.tile([128, 8], f32, tag="rec")
                nc.vector.reciprocal(rec[:], deg2[:])
                row = row_replicate_pre(rec, "rd")
                if s < 2 or s >= BLOC - 2:
                    v["drep"] = row_replicate_psum(row, "rd")
                else:
                    v["drep"] = row_replicate_dma(row, s, 1, "rd")

            def stage_stats(s):
                v = st[s]
                pT, drep, fTh = v["pT"], v["drep"], v["fTh"]
                # c_col = colsum of f16h over Act tiles = accum of fT_h cols
                c_col = smpool.tile([128, 1], f32, tag="ccol")
                cscr = gpool.tile([128, NA * 128], bf16, tag="cscr")
                nc.scalar.activation(
                    cscr[:], fTh[:, 0 : NA * 128], AF.Copy, accum_out=c_col[:]
                )
                scol = statall[:, s * SW + 8 : s * SW + 9]
                pcolA = statall[:, s * SW + 9 : s * SW + 10]
                pcolB = statall[:, s * SW + 10 : s * SW + 11]
                gcol = statall[:, s * SW + 11 : s * SW + 12]
                # protoS = (pT + c_col) * rrep(2/deg2), accum -> gtsum
                protoS = pspool.tile([128, P], bf16, tag="protoS")
                nc.vector.scalar_tensor_tensor(
                    protoS[:], pT[:], c_col[:], drep[:],
                    op0=OP.add, op1=OP.mult,
                    accum_out=gcol,
                )
                # pf2 = sum protoS * fT_h  (pf = 2*pf2)
                g1 = gpool.tile([128, P], bf16, tag="g1")
                nc.vector.scalar_tensor_tensor(
                    g1[:], protoS[:], 1.0, fTh[:],
                    op0=OP.mult, op1=OP.mult,
                    accum_out=scol,
                )
                # pp = sum protoS^2, split across Act and DVE halves
                g2 = gpool.tile([128, P], bf16, tag="g2")
                nc.scalar.activation(
                    g2[:, 0:512], protoS[:, 0:512], AF.Square, accum_out=pcolA
                )
                nc.vector.scalar_tensor_tensor(
                    g2[:, 512:P], protoS[:, 512:P], 1.0, protoS[:, 512:P],
                    op0=OP.mult, op1=OP.mult,
                    accum_out=pcolB,
                )
                del st[s]

            # software pipeline
            for k in range(BLOC + 3):
                if k < BLOC:
                    stage_load(k)
                if 1 <= k <= BLOC:
                    stage_prep(k - 1)
                if 3 <= k <= BLOC + 2:
                    stage_stats(k - 3)
                if 2 <= k <= BLOC + 1:
                    stage_simx_head(k - 2)
                    stage_sim(k - 2)

            nc.sync.dma_start(out_d[:], statall[:])

    nc.compile()
    return nc


def _get_program():
    global _PROG
    if _PROG is None:
        _PROG = _build_program()
    return _PROG


def _host_reduce(stats: np.ndarray) -> np.float32:
    """stats: [B, 128, SW] per-sample device stats -> scalar loss."""
    stats = stats.astype(np.float64)
    N = float(P * D)
    ff = stats[:, :, 0:8].sum(axis=(1, 2))       # Sum f^2
    pf = 2.0 * stats[:, :, 8].sum(axis=1)        # Sum protoS*f
    pp = (stats[:, :, 9] + stats[:, :, 10]).sum(axis=1)  # Sum protoS^2
    gtsum = stats[:, :, 11]                      # [B, D] Sum_q protoS

    mse = (pp - 2.0 * pf + ff) / N
    sum_proto = gtsum.sum(axis=1)
    gtm = gtsum / float(P)
    ybar = sum_proto / N
    S = ((gtm - ybar[:, None]) ** 2).sum(axis=1)
    sum_xc2 = pp - (sum_proto ** 2) / N
    num = float(P) * S
    corr = num / np.sqrt(sum_xc2 * num)
    loss = mse.mean() + (0.5 * (corr + 1.0)).mean()
    return np.float32(loss)


_LAST_RESULTS = None


def kernel(features: np.ndarray, tau: np.ndarray, **run_kwargs) -> np.ndarray:
    global _LAST_RESULTS
    from concourse import bass_utils

    features = np.ascontiguousarray(features, dtype=np.float32)
    tau_v = np.array(tau, dtype=np.float32).reshape(1, 1)

    nc = _get_program()
    shards = features.reshape(NCORES, BLOC, P, D)
    oh = np.zeros((8, NT * 128), dtype=np.float32)
    for t in range(NT):
        oh[t, t * 128 : (t + 1) * 128] = 1.0
    import ml_dtypes
    oh = oh.astype(ml_dtypes.bfloat16)
    in_maps = [
        {"features": shards[i], "tau": tau_v.copy(), "onehot": oh.copy()}
        for i in range(NCORES)
    ]
    res = bass_utils.run_bass_kernel_spmd(
        nc, in_maps, core_ids=list(range(NCORES)), **run_kwargs
    )
    _LAST_RESULTS = res
    stats = np.concatenate(
        [
            res.results[i]["out"].reshape(128, BLOC, SW).transpose(1, 0, 2)
            for i in range(NCORES)
        ],
        axis=0,
    )
    return _host_reduce(stats)


if __name__ == "__main__":
    x = np.random.randn(B, P, D).astype(np.float32)
    t = np.float32(0.5)
    print(kernel(x, t))


# revision 3
# speedup vs baseline: 1.0240x; 1.0240x over previous
"""Trainium2 Bass kernel for nn_CcLoss (gnn_message_passing) — v2.

Full inputs: features [64, 1024, 128] f32, tau scalar f32.
Data-parallel over batch B across 8 NeuronCores (8 samples per core).
~150us HW exec (2.5x faster than the v1 fp8/Pool-based kernel at 378us),
rel err ~1.3e-5 vs the f64 reference reduction.

Design (per sample s on a core):
  f16h  = f/2 in bf16            (DVE tensor_scalar, 2x mode)
  ff    = per-row |f|^2          (Act Square + DVE per-tile reduce)
  r     = sqrt(ff) (Act), rinv = 1/r (DVE reciprocal)
  fT_h  = transpose(f16h): staged to DRAM, then ONE dma_start_transpose
          ([1024,128] -> [128,1024], 64 XBAR tiles in a single SP instr —
          ~8x cheaper in SP time than 8 per-tile SBUF transposes)
  rrep  = 2*rinv broadcast along partitions (PE f32 transpose + DVE copy +
          DRAM row round-trip; first/last samples use an on-chip PE one-hot
          K=8 broadcast into PSUM + Act copy to avoid DMA latency in the
          pipeline fill/drain)
  fnT   = fT_h * rrep  (= fn transposed; one-sided normalization)
  sim   = fT_h.T @ fnT per row-tile (PE bf16, out = sim * r_p / 2) and
          thresholded against taur_p = tau*r_p/2 (per-partition):
            tiles 0..NA-1 on Act:  S = Sign(psum - taur) in {-1,+1}
            tiles NA..7  on DVE:   is_gt * 2 in {0,2} (via scalar_tensor_
            tensor with a const-2 tensor; the dual-immediate tensor_scalar
            path miscomputes elements near 2*taur on HW)
          accum_out gives dacc; deg2 = 2*deg = dacc (+1024 on Sign tiles)
  protoT= sum_k f16h_k.T @ mask2_k (PE bf16 x bf16, one contiguous
          16-matmul accumulation group; mask bf16 — no fp8 split needed)
  protoS= (protoT + c_col) * rep(2/deg2)   (DVE stt, accum -> gtsum)
          c_col corrects the Sign-form tiles: colsum of f16h over them.
  stats : pf2 = sum protoS*fT_h (DVE stt accum), pp split across
          Act Square / DVE stt halves. All stats accumulate directly into
          the output tile; host combines them into the exact MSE + Pearson
          algebra of the reference.

Pipeline: 3-deep software pipeline (load | prep | transpose+sim | stats),
scheduled by the tile framework; activation tables (Square/Sqrt/Sign) are
prefetched at program start so their ~2.7us loads stay out of the fill.
"""

import numpy as np

B, P, D = 64, 1024, 128
NCORES = 8
BLOC = B // NCORES          # samples per core
NT = P // 128               # 128-row tiles per sample
SW = 12                     # stat cols: 0:8 ff_t, 8 pf2, 9 ppA, 10 ppB, 11 gt

# Threshold engine split: sim row-tiles 0..NA-1 on Act (Sign form),
# NA..7 on DVE (is_gt * 2 -> {0,2}).
NA = 5
# sim row-tile processing order interleaves Act/DVE thresholds
MT_ORDER = (0, 5, 1, 6, 2, 7, 3, 4)
# which engine materializes pp (Square protoS): "dve" or "act"
PP_ENGINE = "dve"

_PROG = None


def _build_program():
    import concourse.tile as tile
    from concourse import bacc, mybir, masks

    f32 = mybir.dt.float32
    bf16 = mybir.dt.bfloat16
    AF = mybir.ActivationFunctionType
    OP = mybir.AluOpType

    nc = bacc.Bacc(
        "TRN2",
        target_bir_lowering=False,
        debug=False,
        enable_asserts=False,
        num_devices=NCORES,
    )
    feats = nc.dram_tensor("features", [BLOC, P, D], f32, kind="ExternalInput").ap()
    tau_d = nc.dram_tensor("tau", [1, 1], f32, kind="ExternalInput").ap()
    oh_d = nc.dram_tensor("onehot", [8, NT * 128], bf16, kind="ExternalInput").ap()
    out_d = nc.dram_tensor("out", [128, BLOC * SW], f32, kind="ExternalOutput").ap()
    # DRAM staging: f16h per sample (p-major) for the big XBAR transpose
    f16d = nc.dram_tensor("f16_scratch", [BLOC, P, D], bf16, kind="Internal").ap()
    # row scratch for partition-replication round trips (rinv2 / rdeg2)
    rrow_d = nc.dram_tensor("rrow_scratch", [BLOC, 2, P], bf16, kind="Internal").ap()

    with tile.TileContext(nc) as tc:
        from contextlib import ExitStack

        with ExitStack() as ctx:
            const = ctx.enter_context(tc.tile_pool(name="const", bufs=1))
            fpool = ctx.enter_context(tc.tile_pool(name="f", bufs=3))
            sqpool = ctx.enter_context(tc.tile_pool(name="sq", bufs=2))
            fhpool = ctx.enter_context(tc.tile_pool(name="f16h", bufs=5))
            ftpool = ctx.enter_context(tc.tile_pool(name="fTh", bufs=4))
            fnpool = ctx.enter_context(tc.tile_pool(name="fnT", bufs=3))
            mpool = ctx.enter_context(tc.tile_pool(name="mask", bufs=2))
            pspool = ctx.enter_context(tc.tile_pool(name="protoS", bufs=2))
            gpool = ctx.enter_context(tc.tile_pool(name="gscr", bufs=4))
            smpool = ctx.enter_context(tc.tile_pool(name="small", bufs=8))
            rowpool = ctx.enter_context(tc.tile_pool(name="rows", bufs=4))
            reppool = ctx.enter_context(tc.tile_pool(name="reps", bufs=4))
            pss_pool = ctx.enter_context(tc.tile_pool(name="pss", bufs=2, space="PSUM"))
            pT_pool = ctx.enter_context(tc.tile_pool(name="pT", bufs=1, space="PSUM"))
            prow_pool = ctx.enter_context(tc.tile_pool(name="prow", bufs=2, space="PSUM"))

            ident16 = const.tile([128, 128], bf16)
            masks.make_identity(nc, ident16[:])
            ident32 = const.tile([128, 128], f32)
            masks.make_identity(nc, ident32[:])
            tau_bc = const.tile([128, 1], f32)
            nc.sync.dma_start(tau_bc[:], tau_d[0, :].partition_broadcast(128))
            statall = const.tile([128, BLOC * SW], f32)
            twos = const.tile([128, P], bf16)
            nc.vector.memset(twos[:], 2.0)
            # one-hot rows for PE row-broadcast (fill/drain path)
            onehot = const.tile([8, NT * 128], bf16)
            nc.sync.dma_start(onehot[:], oh_d)
            # prefetch activation tables (Square/Sqrt/Sign) so the ~2.7us
            # table loads don't land mid-pipeline
            warm = const.tile([128, 1], f32)
            nc.scalar.activation(warm[:], tau_bc[:], AF.Square)
            nc.scalar.activation(warm[:], tau_bc[:], AF.Sqrt)
            nc.scalar.activation(warm[:], tau_bc[:], AF.Sign)

            st = {}

            def row_replicate_pre(src32, tag):
                """[128,8] f32 -> [8,128] bf16 row of 2*src via PE f32
                transpose + small DVE copy (no Act in the chain)."""
                prow = prow_pool.tile([8, 128], f32, tag="rowT")
                nc.tensor.matmul(prow[:], src32[:], ident32[:], is_transpose=True)
                row8 = rowpool.tile([8, 128], bf16, tag=f"{tag}r8")
                nc.vector.tensor_scalar_mul(row8[:], prow[:], 2.0)
                return row8

            def row_replicate_psum(row8, tag):
                """[8,128] bf16 row -> [128,P] bf16 replicated via 8 K=8
                one-hot matmuls into PSUM + Act copy. No DRAM latency; used
                for fill/drain samples where engines are otherwise idle."""
                rps = pss_pool.tile([128, P], f32, tag="pss")
                for t in range(NT):
                    nc.tensor.matmul(
                        rps[:, t * 128 : (t + 1) * 128],
                        onehot[:, t * 128 : (t + 1) * 128],
                        row8[:],
                        start=True,
                        stop=True,
                    )
                rep = reppool.tile([128, P], bf16, tag=f"{tag}rep")
                nc.scalar.copy(rep[:], rps[:])
                return rep

            def row_replicate_dma(row8, s, slot, tag):
                """Store the row to DRAM and broadcast-load along
                partitions (emitted late so SP never waits on it)."""
                nc.sync.dma_start(
                    rrow_d[s, slot].rearrange("(t p) -> t p", t=NT), row8[:]
                )
                rep = reppool.tile([128, P], bf16, tag=f"{tag}rep")
                nc.sync.dma_start(rep[:], rrow_d[s, slot].partition_broadcast(128))
                return rep

            def stage_load(s):
                fb = fpool.tile([128, NT * 128], f32, tag="fb")
                nc.sync.dma_start(
                    fb[:].rearrange("p (t d) -> p t d", t=NT),
                    feats[s].rearrange("(t p) d -> p t d", p=128),
                )
                st[s] = {"fb": fb}

            def stage_prep(s):
                v = st[s]
                fb = v["fb"]
                ffcols = statall[:, s * SW : s * SW + 8]
                # row norms^2 per tile: Act Square + DVE per-tile reduce
                sq = sqpool.tile([128, NT * 128], f32, tag="sq")
                nc.scalar.activation(sq[:], fb[:], AF.Square)
                nc.vector.tensor_reduce(
                    ffcols,
                    sq[:].rearrange("p (t d) -> p t d", t=NT),
                    axis=mybir.AxisListType.X,
                    op=OP.add,
                )
                sroot = smpool.tile([128, 8], f32, tag="sroot")
                nc.scalar.activation(sroot[:], ffcols, AF.Sqrt)
                rinv = smpool.tile([128, 8], f32, tag="rinv")
                nc.vector.reciprocal(rinv[:], sroot[:])
                # taur = (tau/2)*r ; ntaur = -(tau/2)*r   [Pool]
                taur = smpool.tile([128, 8], f32, tag="taur")
                nc.gpsimd.tensor_scalar(
                    taur[:], sroot[:], tau_bc[:], 0.5, op0=OP.mult, op1=OP.mult
                )
                ntaur = smpool.tile([128, 8], f32, tag="ntaur")
                nc.gpsimd.tensor_scalar_mul(ntaur[:], taur[:], -1.0)

                # f16h = 0.5*f in bf16 (DVE, 2x_2P) and stage to DRAM
                f16h = fhpool.tile([128, NT * 128], bf16, tag="f16h")
                nc.vector.tensor_scalar_mul(f16h[:], fb[:], 0.5)
                nc.sync.dma_start(
                    f16d[s].rearrange("(t p) d -> p t d", p=128),
                    f16h[:].rearrange("p (t d) -> p t d", t=NT),
                )
                row = row_replicate_pre(rinv, "ri")
                if s < 2:
                    v["rrep"] = row_replicate_psum(row, "ri")
                else:
                    v["rrep"] = row_replicate_dma(row, s, 0, "ri")
                v.update(f16h=f16h, taur=taur, ntaur=ntaur)

            def stage_simx_head(s):
                v = st[s]
                # one big XBAR transpose from DRAM: [P, D] -> [128(d), P]
                fTh = ftpool.tile([128, P], bf16, tag="fTh")
                nc.sync.dma_start_transpose(fTh[:], f16d[s])
                # fnT = fT_h * rinv2_rep  (bf16 TT)
                fnT = fnpool.tile([128, P], bf16, tag="fnT")
                nc.vector.tensor_tensor(
                    fnT[:], fTh[:], v["rrep"][:], op=OP.mult
                )
                v.update(fTh=fTh, fnT=fnT)

            def stage_sim(s):
                v = st[s]
                fTh, fnT = v["fTh"], v["fnT"]
                taur, ntaur = v["taur"], v["ntaur"]
                mask_t = mpool.tile([128, NT * P], bf16, tag="mask")
                dacc = smpool.tile([128, 8], f32, tag="dacc")
                pT = pT_pool.tile([128, P], f32, tag="pT")
                v["pT"] = pT
                mk = mask_t[:].rearrange("p (k q) -> p k q", k=NT)
                f16h = v["f16h"].rearrange("p (k d) -> p k d", k=NT)
                for mt in MT_ORDER:
                    pss = pss_pool.tile([128, 1024], f32, tag="pss")
                    for nb in range(2):
                        nc.tensor.matmul(
                            pss[:, nb * 512 : (nb + 1) * 512],
                            fTh[:, mt * 128 : (mt + 1) * 128],
                            fnT[:, nb * 512 : (nb + 1) * 512],
                            start=True,
                            stop=True,
                        )
                    blk = mask_t[:, mt * P : (mt + 1) * P]
                    if mt < NA:
                        # S in {-1,+1}; dacc = 2*deg - 1024
                        nc.scalar.activation(
                            blk, pss[:], AF.Sign, bias=ntaur[:, mt : mt + 1],
                            accum_out=dacc[:, mt : mt + 1],
                        )
                    else:
                        # {0,2}; dacc = 2*deg  (stt with const-2 tensor: the
                        # dual-imm tensor_scalar path miscomputes max elements)
                        nc.vector.scalar_tensor_tensor(
                            blk, pss[:], taur[:, mt : mt + 1], twos[:],
                            op0=OP.is_gt, op1=OP.mult,
                            accum_out=dacc[:, mt : mt + 1],
                        )
                # protoT accumulation as one contiguous PE group: proto-k
                # only needs mask-k, all long since ready -> dense MM burst
                for done, mt in enumerate(MT_ORDER):
                    for nb in range(2):
                        nc.tensor.matmul(
                            pT[:, nb * 512 : (nb + 1) * 512],
                            f16h[:, mt, :],
                            mk[:, mt, nb * 512 : (nb + 1) * 512],
                            start=(done == 0),
                            stop=(done == 7),
                            skip_group_check=True,
                        )

                # deg2 = 2*deg: Act tiles need +1024  [Pool]
                deg2 = smpool.tile([128, 8], f32, tag="deg2")
                nc.gpsimd.tensor_scalar(
                    deg2[:, 0:NA], dacc[:, 0:NA], 1.0, 1024.0,
                    op0=OP.mult, op1=OP.add,
                )
                nc.gpsimd.tensor_copy(deg2[:, NA:8], dacc[:, NA:8])
                rec = smpool.tile([128, 8], f32, tag="rec")
                nc.vector.reciprocal(rec[:], deg2[:])
                row = row_replicate_pre(rec, "rd")
                if s < 2 or s >= BLOC - 2:
                    v["drep"] = row_replicate_psum(row, "rd")
                else:
                    v["drep"] = row_replicate_dma(row, s, 1, "rd")

            def stage_stats(s):
                v = st[s]
                pT, drep, fTh = v["pT"], v["drep"], v["fTh"]
                # c_col = colsum of f16h over Act tiles = accum of fT_h cols
                c_col = smpool.tile([128, 1], f32, tag="ccol")
                cscr = gpool.tile([128, NA * 128], bf16, tag="cscr")
                nc.scalar.activation(
                    cscr[:], fTh[:, 0 : NA * 128], AF.Copy, accum_out=c_col[:]
                )
                scol = statall[:, s * SW + 8 : s * SW + 9]
                pcolA = statall[:, s * SW + 9 : s * SW + 10]
                pcolB = statall[:, s * SW + 10 : s * SW + 11]
                gcol = statall[:, s * SW + 11 : s * SW + 12]
                # protoS = (pT + c_col) * rrep(2/deg2), accum -> gtsum
                protoS = pspool.tile([128, P], bf16, tag="protoS")
                nc.vector.scalar_tensor_tensor(
                    protoS[:], pT[:], c_col[:], drep[:],
                    op0=OP.add, op1=OP.mult,
                    accum_out=gcol,
                )
                # pf2 = sum protoS * fT_h  (pf = 2*pf2)
                g1 = gpool.tile([128, P], bf16, tag="g1")
                nc.vector.scalar_tensor_tensor(
                    g1[:], protoS[:], 1.0, fTh[:],
                    op0=OP.mult, op1=OP.mult,
                    accum_out=scol,
                )
                # pp = sum protoS^2, split across Act and DVE halves
                g2 = gpool.tile([128, P], bf16, tag="g2")
                nc.scalar.activation(
                    g2[:, 0:512], protoS[:, 0:512], AF.Square, accum_out=pcolA
                )
                nc.vector.scalar_tensor_tensor(
                    g2[:, 512:P], protoS[:, 512:P], 1.0, protoS[:, 512:P],
                    op0=OP.mult, op1=OP.mult,
                    accum_out=pcolB,
                )
                del st[s]

            # software pipeline
            for k in range(BLOC + 3):
                if k < BLOC:
                    stage_load(k)
                if 1 <= k <= BLOC:
                    stage_prep(k - 1)
                if 3 <= k <= BLOC + 2:
                    stage_stats(k - 3)
                if 2 <= k <= BLOC + 1:
                    stage_simx_head(k - 2)
                    stage_sim(k - 2)

            nc.sync.dma_start(out_d[:], statall[:])


    nc.compile()
    return nc


def _get_program():
    global _PROG
    if _PROG is None:
        _PROG = _build_program()
    return _PROG


def _host_reduce(stats: np.ndarray) -> np.float32:
    """stats: [B, 128, SW] per-sample device stats -> scalar loss."""
    stats = stats.astype(np.float64)
    N = float(P * D)
    ff = stats[:, :, 0:8].sum(axis=(1, 2))       # Sum f^2
    pf = 2.0 * stats[:, :, 8].sum(axis=1)        # Sum protoS*f
    pp = (stats[:, :, 9] + stats[:, :, 10]).sum(axis=1)  # Sum protoS^2
    gtsum = stats[:, :, 11]                      # [B, D] Sum_q protoS

    mse = (pp - 2.0 * pf + ff) / N
    sum_proto = gtsum.sum(axis=1)
    gtm = gtsum / float(P)
    ybar = sum_proto / N
    S = ((gtm - ybar[:, None]) ** 2).sum(axis=1)
    sum_xc2 = pp - (sum_proto ** 2) / N
    num = float(P) * S
    corr = num / np.sqrt(sum_xc2 * num)
    loss = mse.mean() + (0.5 * (corr + 1.0)).mean()
    return np.float32(loss)


_LAST_RESULTS = None


def kernel(features: np.ndarray, tau: np.ndarray, **run_kwargs) -> np.ndarray:
    global _LAST_RESULTS
    from concourse import bass_utils

    features = np.ascontiguousarray(features, dtype=np.float32)
    tau_v = np.array(tau, dtype=np.float32).reshape(1, 1)

    nc = _get_program()
    shards = features.reshape(NCORES, BLOC, P, D)
    oh = np.zeros((8, NT * 128), dtype=np.float32)
    for t in range(NT):
        oh[t, t * 128 : (t + 1) * 128] = 1.0
    import ml_dtypes
    oh = oh.astype(ml_dtypes.bfloat16)
    in_maps = [
        {"features": shards[i], "tau": tau_v.copy(), "onehot": oh.copy()}
        for i in range(NCORES)
    ]
    res = bass_utils.run_bass_kernel_spmd(
        nc, in_maps, core_ids=list(range(NCORES)), **run_kwargs
    )
    _LAST_RESULTS = res
    stats = np.concatenate(
        [
            res.results[i]["out"].reshape(128, BLOC, SW).transpose(1, 0, 2)
            for i in range(NCORES)
        ],
        axis=0,
    )
    return _host_reduce(stats)


if __name__ == "__main__":
    x = np.random.randn(B, P, D).astype(np.float32)
    t = np.float32(0.5)
    print(kernel(x, t))
